# revision 2
# baseline (speedup 1.0000x reference)
"""Multi-head attention (B=2, M=N=2048, D=1024, H=16, DH=64) on 8 TRN2 cores.

Sharding: data-parallel over batch (cores 0-3 = batch 0, 4-7 = batch 1),
tensor-parallel over heads within each batch group (4 heads/core).

v2 design (vs v1): x is transposed on the HOST (kills all PE transposes
and their PSUM->SBUF copies), stage 1 streams x in 512-column blocks on
three DMA queues (sync=xk, vector=xq, gpsimd=weights+xv) so the first
S^T lands ~13us in; stage-1 projection work (kT blocks 1-3, qT m1-3, all
V projections) is interleaved into the (m0,p0) attention loop to fill PE
slack under the ScalarE exp stream.

Attention per (m-chunk 512, head-pair p, n-tile): S^T = K_h Q_h^T row-
tiled pair into one 2-bank PSUM tile; one exp -> bf16 `at` (A^T layout);
AV in one of two dataflows:
  - stat_at: at chunks [n128, m128] are the matmul stationary, moving =
    v_sb[n128, 65] = [V_h | ones] bf16 -> O[m-part, dh] accumulates in
    PSUM with the softmax denominator in col 64. 65-row moving streams
    (bf16 is 1 cycle/row at any free size) halve AV PE rows vs v1;
    normalize is a per-partition DVE reciprocal + tensor_scalar multiply
    (no gpsimd broadcast); O is PE-transposed (bf16 identity) to O^T
    before the AllGather.
  - stat_v: v1 dataflow ([ones|0|V] stationary, at moving, O^T direct).
exp can optionally be split ScalarE/DVE: DVE computes a Schraudolph-
style exp (i16 = round(x*184.665 + B); bitcast bf16) in one
tensor_scalar pass - relative sawtooth error ~3% on attention weights,
which cancels to <1% after softmax normalization + output projection.

Per-m-chunk (256,512) bf16 O^T shards AllGather across the 4-core batch
group; each core computes a 256-wide output-channel slice of out^T =
Wo_slice^T.T @ O^T_full per m-chunk (emitted last => lowest PE priority,
fills PE gaps). Host-side prep: weights pre-transposed/sliced per core,
bv folded into bo_eff = bo + Wo @ bv, bk dropped (cancels in softmax).
"""

import os

import numpy as np

B, M, NSEQ, D = 2, 2048, 2048, 1024
H, DH = 16, 64
HC = 4                # heads per core
PC = HC * DH          # 256 projected channels per core
CT = D // 128         # 8 contraction tiles
NT = NSEQ // 128      # 16 n-tiles
MT = M // 512         # 4 m-chunks
NB = 4                # 512-col x blocks
NCORES = 8

AV_MODE = os.environ.get("AV_MODE", "stat_at")       # stat_at | stat_v
EXP_SPLIT = float(os.environ.get("EXP_SPLIT", "0"))  # fraction of exp tiles on DVE
SCHRAUDOLPH_A = 184.6650
SCHRAUDOLPH_B = float(os.environ.get("SCHRAUDOLPH_B", "16249.0"))

_CACHE = {}


def _build(single_core=False, reps=1, av_mode=None, exp_split=None):
    import concourse.bass as bass
    import concourse.tile as tile
    from concourse import bacc, mybir
    from concourse.masks import make_identity

    av_mode = AV_MODE if av_mode is None else av_mode
    exp_split = EXP_SPLIT if exp_split is None else exp_split

    F32 = mybir.dt.float32
    F32R = mybir.dt.float32r
    BF16 = mybir.dt.bfloat16
    I16 = mybir.dt.int16
    AF = mybir.ActivationFunctionType

    nc = bacc.Bacc(
        "TRN2",
        target_bir_lowering=False,
        debug=False,
        num_devices=1 if single_core else 8,
    )

    xqT_d = nc.dram_tensor("xqT", [D, M], F32, kind="ExternalInput")
    xkT_d = nc.dram_tensor("xkT", [D, NSEQ], F32, kind="ExternalInput")
    xvT_d = nc.dram_tensor("xvT", [D, NSEQ], BF16, kind="ExternalInput")
    wq_d = nc.dram_tensor("wq", [D, PC], F32, kind="ExternalInput")
    wk_d = nc.dram_tensor("wk", [D, PC], F32, kind="ExternalInput")
    wv_d = nc.dram_tensor("wv", [D, PC], BF16, kind="ExternalInput")
    wo_d = nc.dram_tensor("wo", [D, PC], BF16, kind="ExternalInput")
    bq_d = nc.dram_tensor("bq", [PC, 1], F32, kind="ExternalInput")
    bo_d = nc.dram_tensor("bo", [PC, 1], F32, kind="ExternalInput")
    outT_d = nc.dram_tensor("outT", [PC, M], F32, kind="ExternalOutput")
    debug = bool(int(os.environ.get("KERNEL_DEBUG", "0")))
    dbg = None
    if debug:
        dbg = dict(
            qT_dbg=nc.dram_tensor("qT_dbg", [128, 2, M], F32, kind="ExternalOutput"),
            kT_dbg=nc.dram_tensor("kT_dbg", [128, 2, NSEQ], F32, kind="ExternalOutput"),
            v_dbg=nc.dram_tensor("v_dbg", [128, NT, HC, 65], BF16, kind="ExternalOutput"),
            agin_dbg=nc.dram_tensor("agin_dbg", [MT, PC, 512], BF16, kind="ExternalOutput"),
            agout_dbg=nc.dram_tensor("agout_dbg", [MT, 4 * PC, 512], BF16, kind="ExternalOutput"),
        )

    with tile.TileContext(nc) as tc:
        with (
            tc.tile_pool(name="singles", bufs=1) as singles,
            tc.tile_pool(name="dram", bufs=1, space="DRAM") as dram,
        ):
            ident = singles.tile([128, 128], BF16)
            make_identity(nc, ident)
            bq_sb = singles.tile([128, 2], F32)
            nc.gpsimd.dma_start(
                out=bq_sb, in_=bq_d[:, :].rearrange("(o p) w -> p (o w)", p=128)
            )
            bo_sb = singles.tile([128, 2], F32)
            nc.gpsimd.dma_start(
                out=bo_sb, in_=bo_d[:, :].rearrange("(o p) w -> p (o w)", p=128)
            )

            ag_in = dram.tile([MT, PC, 512], BF16)
            ag_out = dram.tile([MT, 4 * PC, 512], BF16)

            env = dict(
                xqT_d=xqT_d, xkT_d=xkT_d, xvT_d=xvT_d,
                wq_d=wq_d, wk_d=wk_d, wv_d=wv_d, wo_d=wo_d,
                outT_d=outT_d, ident=ident, bq_sb=bq_sb, bo_sb=bo_sb,
                ag_in=ag_in, ag_out=ag_out,
            )
            for rep in range(reps):
                _emit_rep(
                    nc, tc, bass, mybir, F32, F32R, BF16, I16, AF,
                    rep, single_core, av_mode, exp_split,
                    dbg if (debug and rep == reps - 1) else None, env,
                )
    nc.compile()
    return nc


def _emit_rep(nc, tc, bass, mybir, F32, F32R, BF16, I16, AF, rep,
              single_core, av_mode, exp_split, dbg, env):
    ident = env["ident"]
    bq_sb, bo_sb = env["bq_sb"], env["bo_sb"]
    ag_in, ag_out = env["ag_in"], env["ag_out"]
    R = f"r{rep}_"
    stat_at = av_mode == "stat_at"

    with (
        tc.tile_pool(name=f"{R}persist", bufs=1) as persist,
        tc.tile_pool(name=f"{R}ps_s", bufs=2, space="PSUM") as ps_s_pool,
        tc.tile_pool(name=f"{R}ps_av", bufs=1 if stat_at else 2,
                     space="PSUM") as ps_av_pool,
        tc.tile_pool(name=f"{R}at", bufs=6) as at_pool,
        tc.tile_pool(name=f"{R}onat", bufs=8) as onat_pool,
        tc.tile_pool(name=f"{R}otb", bufs=2) as otb_pool,
        tc.tile_pool(name=f"{R}rec", bufs=4) as rec_pool,
        tc.tile_pool(name=f"{R}og", bufs=2) as og_pool,
        tc.tile_pool(name=f"{R}osb", bufs=2) as osb_pool,
    ):
        kT = persist.tile([128, 2, M], F32R, name=f"{R}kT")
        qT = persist.tile([128, 2, M], F32R, name=f"{R}qT")
        wk_sb = persist.tile([128, CT, PC], F32R, name=f"{R}wk_sb")
        wq_sb = persist.tile([128, CT, PC], F32R, name=f"{R}wq_sb")
        wv_sb = persist.tile([128, CT, PC], BF16, name=f"{R}wv_sb")
        wo_sb = persist.tile([128, CT, PC], BF16, name=f"{R}wo_sb")
        if stat_at:
            # moving operand of AV: [V_h | ones] per (nt, head)
            v_sb = persist.tile([128, NT, HC, 65], BF16, name=f"{R}v_sb")
        else:
            # v1 layout: [ones | zeros*63 | V] stationary
            v_sb = persist.tile([128, HC, NT, 128], BF16, name=f"{R}v_sb")

        # ---- input DMAs -------------------------------------------------
        # gpsimd queue: weights (k first), then xv blocks, then wo
        nc.gpsimd.dma_start(
            out=wk_sb, in_=env["wk_d"][:, :].rearrange("(ct p) c -> p ct c", p=128))
        nc.gpsimd.dma_start(
            out=wq_sb, in_=env["wq_d"][:, :].rearrange("(ct p) c -> p ct c", p=128))
        nc.gpsimd.dma_start(
            out=wv_sb, in_=env["wv_d"][:, :].rearrange("(ct p) c -> p ct c", p=128))

        with (
            tc.tile_pool(name=f"{R}xst", bufs=3) as xst_pool,
            tc.tile_pool(name=f"{R}xvst", bufs=2) as xvst_pool,
        ):
            xv_st = []
            for b in range(NB):
                t = xvst_pool.tile([128, CT, 512], BF16, tag="xv",
                                   name=f"{R}xv{b}")
                nc.gpsimd.dma_start(
                    out=t,
                    in_=env["xvT_d"][:, :].rearrange(
                        "(ct p) n -> p ct n", p=128)[:, :, b * 512:(b + 1) * 512])
                xv_st.append(t)
            nc.gpsimd.dma_start(
                out=wo_sb,
                in_=env["wo_d"][:, :].rearrange("(ct p) c -> p ct c", p=128))

            # sync queue: xk blocks; vector queue: xq blocks
            xk_st, xq_st = [], []
            for b in range(NB):
                t = xst_pool.tile([128, CT, 512], F32R, tag="xk",
                                  name=f"{R}xk{b}")
                nc.sync.dma_start(
                    out=t,
                    in_=env["xkT_d"][:, :].rearrange(
                        "(ct p) n -> p ct n", p=128)[:, :, b * 512:(b + 1) * 512])
                xk_st.append(t)
            for b in range(NB):
                t = xst_pool.tile([128, CT, 512], F32R, tag="xq",
                                  name=f"{R}xq{b}")
                nc.vector.dma_start(
                    out=t,
                    in_=env["xqT_d"][:, :].rearrange(
                        "(ct p) m -> p ct m", p=128)[:, :, b * 512:(b + 1) * 512])
                xq_st.append(t)

            if stat_at:
                nc.gpsimd.memset(v_sb[:, :, :, 64:65], 1.0)
            else:
                nc.gpsimd.memset(v_sb[:, :, :, 0:1], 1.0)
                nc.gpsimd.memset(v_sb[:, :, :, 1:64], 0.0)

            # ---- stage-1 emission helpers ------------------------------
            with tc.tile_pool(name=f"{R}pj", bufs=2, space="PSUM") as pj_pool:

                def proj_qk(dst, w_sb, x_st, b, is_q):
                    for ot in range(2):
                        pj = pj_pool.tile([128, 512], F32, tag="pj",
                                          name=f"{R}pj{b}_{ot}_{int(is_q)}")
                        for ct in range(CT):
                            nc.tensor.matmul(
                                pj,
                                w_sb[:, ct, ot * 128:(ot + 1) * 128],
                                x_st[:, ct, :],
                                start=(ct == 0), stop=(ct == CT - 1))
                        d = dst[:, ot, b * 512:(b + 1) * 512]
                        if is_q:
                            nc.vector.tensor_scalar_add(
                                d, pj, bq_sb[:, ot:ot + 1])
                        else:
                            nc.vector.tensor_copy(d, pj)

                def proj_v(nt):
                    vb, s = nt // 4, nt % 4
                    psv = pj_pool.tile([128, 512], F32, tag="pj",
                                       name=f"{R}psv{nt}")
                    for ct in range(CT):
                        nc.tensor.matmul(
                            psv[:, 0:PC],
                            xv_st[vb][:, ct, s * 128:(s + 1) * 128],
                            wv_sb[:, ct, :],
                            start=(ct == 0), stop=(ct == CT - 1))
                    if stat_at:
                        nc.vector.tensor_copy(
                            v_sb[:, nt, :, 0:64],
                            psv[:, 0:PC].rearrange("p (h d) -> p h d", h=HC))
                    else:
                        nc.vector.tensor_copy(
                            v_sb[:, :, nt, 64:128],
                            psv[:, 0:PC].rearrange("p (h d) -> p h d", h=HC))

                # ---- stage-2 emission helpers --------------------------
                def st_exp(m, p, nt, on_dve):
                    ps_s = ps_s_pool.tile([128, 1024], F32, tag="pss",
                                          name=f"{R}pss{m}_{p}_{nt}")
                    for j in range(2):
                        base = j * 64
                        nc.tensor.matmul(
                            ps_s[:, j * 512:(j + 1) * 512],
                            kT[base:base + 64, p, nt * 128:(nt + 1) * 128],
                            qT[base:base + 64, p, m * 512:(m + 1) * 512],
                            start=True, stop=True)
                    at = at_pool.tile([128, 1024], BF16, tag="at",
                                      name=f"{R}at{m}_{p}_{nt}")
                    if on_dve:
                        nc.vector.tensor_scalar(
                            at.bitcast(I16), ps_s,
                            SCHRAUDOLPH_A, SCHRAUDOLPH_B,
                            mybir.AluOpType.mult, mybir.AluOpType.add)
                    else:
                        nc.scalar.activation(at, ps_s, AF.Exp)
                    return at

                def av_stat_at(m, p, nt, at, av):
                    for j in range(2):
                        for mi in range(4):
                            nc.tensor.matmul(
                                av[:, j * 512 + mi * 65:j * 512 + mi * 65 + 65],
                                at[:, j * 512 + mi * 128:j * 512 + (mi + 1) * 128],
                                v_sb[:, nt, 2 * p + j, :],
                                start=(nt == 0), stop=(nt == NT - 1))

                def av_stat_v(m, p, nt, at, ps_o):
                    for j in range(2):
                        nc.tensor.matmul(
                            ps_o[j],
                            v_sb[:, 2 * p + j, nt, :],
                            at[:, j * 512:(j + 1) * 512],
                            start=(nt == 0), stop=(nt == NT - 1))

                def normalize_stat_at(m, p, av, onat_m):
                    rec = rec_pool.tile([128, 8], F32, tag="rec",
                                        name=f"{R}rec{m}_{p}")
                    nc.vector.reciprocal(
                        rec[:, :].rearrange("p (j i) -> p j i", j=2),
                        av[:, :].rearrange("p (j w) -> p j w", j=2)[:, :, 64::65])
                    for j in range(2):
                        for mi in range(4):
                            nc.vector.tensor_scalar_mul(
                                onat_m[mi][:, (2 * p + j) * 64:(2 * p + j + 1) * 64],
                                av[:, j * 512 + mi * 65:j * 512 + mi * 65 + 64],
                                rec[:, j * 4 + mi:j * 4 + mi + 1])

                def normalize_stat_v(m, p, ps_o):
                    for j in range(2):
                        h = 2 * p + j
                        rec = rec_pool.tile([1, 512], F32, tag="recv",
                                            name=f"{R}rec{m}_{p}_{j}")
                        nc.vector.reciprocal(rec, ps_o[j][0:1, :])
                        rbc = rec_pool.tile([128, 512], F32, tag="rbc",
                                            name=f"{R}rbc{m}_{p}_{j}")
                        nc.gpsimd.partition_broadcast(rbc, rec[0:1, :])
                        osc = rec_pool.tile([64, 512], BF16, tag="osc",
                                            name=f"{R}osc{m}_{p}_{j}")
                        nc.gpsimd.scalar_tensor_tensor(
                            out=osc, in0=ps_o[j][64:128, :], scalar=1.0,
                            in1=rbc[64:128, :],
                            op0=mybir.AluOpType.mult,
                            op1=mybir.AluOpType.mult)
                        nc.vector.dma_start(
                            out=ag_in[m, h * DH:(h + 1) * DH, :], in_=osc)

                def dve_set(m, p):
                    # which nt tiles exp on DVE (evenly spread);
                    # none during (m0,p0) - DVE is busy with stage-1 copies
                    if exp_split <= 0 or (m == 0 and p == 0):
                        return set()
                    d = int(round(NT * exp_split))
                    return {nt for nt in range(NT) if (nt * d) % NT < d}

                # ============ emission ======================================
                proj_qk(kT, wk_sb, xk_st[0], 0, False)
                proj_qk(qT, wq_sb, xq_st[0], 0, True)

                # (m0, p0) with stage-1 work interleaved
                m, p = 0, 0
                if stat_at:
                    av0 = ps_av_pool.tile([128, 1024], F32, tag="av",
                                          name=f"{R}av{m}_{p}")
                else:
                    av0 = [ps_av_pool.tile([128, 512], F32, tag="av",
                                           name=f"{R}av{m}_{p}_{j}")
                           for j in range(2)]
                for nt in range(NT):
                    at = st_exp(m, p, nt, False)
                    proj_v(nt)
                    if stat_at:
                        av_stat_at(m, p, nt, at, av0)
                    else:
                        av_stat_v(m, p, nt, at, av0)
                    if nt in (3, 7, 11):
                        b = nt // 4 + 1
                        proj_qk(kT, wk_sb, xk_st[b], b, False)
                for b in range(1, NB):
                    proj_qk(qT, wq_sb, xq_st[b], b, True)

        # pj pool closed; open aux PSUM pool for transposes + out-proj
        with tc.tile_pool(name=f"{R}aux", bufs=1, space="PSUM") as aux_pool:

            def transpose_out(m, onat_m):
                otb = otb_pool.tile([128, 2, 512], BF16, tag="otb",
                                    name=f"{R}otb{m}")
                for mi in range(4):
                    h2 = mi % 2
                    tr = aux_pool.tile([128, 512], F32, tag="tr",
                                       name=f"{R}tr{m}_{mi // 2}",
                                       bufs=1) if mi % 2 == 0 else tr
                    for g in range(2):
                        nc.tensor.transpose(
                            tr[:, h2 * 256 + g * 128:h2 * 256 + (g + 1) * 128],
                            onat_m[mi][:, g * 128:(g + 1) * 128],
                            ident)
                    nc.vector.tensor_copy(
                        otb[:, :, mi * 128:(mi + 1) * 128],
                        tr[:, h2 * 256:h2 * 256 + 256].rearrange(
                            "p (g w) -> p g w", g=2))
                nc.vector.dma_start(
                    out=ag_in[m].rearrange("(g p) w -> p g w", p=128),
                    in_=otb)

            def gather(m):
                if single_core:
                    for rr in range(4):
                        nc.sync.dma_start(
                            out=ag_out[m, rr * PC:(rr + 1) * PC, :],
                            in_=ag_in[m, :, :])
                else:
                    nc.gpsimd.collective_compute(
                        "AllGather",
                        bass.mybir.AluOpType.bypass,
                        replica_groups=[[0, 1, 2, 3], [4, 5, 6, 7]],
                        ins=[ag_in[m, :, :].opt()],
                        outs=[ag_out[m, :, :].opt()],
                    )

            # finish stage 2
            for m in range(MT):
                onat_m = None
                if stat_at:
                    onat_m = [
                        onat_pool.tile([128, PC], BF16, tag="onat",
                                       name=f"{R}onat{m}_{mi}")
                        for mi in range(4)
                    ]
                for p in range(2):
                    if m == 0 and p == 0:
                        # emitted above inside the pj scope; just normalize
                        if stat_at:
                            normalize_stat_at(m, p, av0, onat_m)
                        else:
                            normalize_stat_v(m, p, av0)
                        continue
                    if stat_at:
                        av = ps_av_pool.tile([128, 1024], F32, tag="av",
                                             name=f"{R}av{m}_{p}")
                    else:
                        av = [ps_av_pool.tile([128, 512], F32, tag="av",
                                              name=f"{R}av{m}_{p}_{j}")
                              for j in range(2)]
                    dset = dve_set(m, p)
                    for nt in range(NT):
                        at = st_exp(m, p, nt, nt in dset)
                        if stat_at:
                            av_stat_at(m, p, nt, at, av)
                        else:
                            av_stat_v(m, p, nt, at, av)
                    if stat_at:
                        normalize_stat_at(m, p, av, onat_m)
                    else:
                        normalize_stat_v(m, p, av)
                if stat_at:
                    transpose_out(m, onat_m)
                gather(m)

            # stage 3: output projection (emitted last => lowest PE priority)
            for m in range(MT):
                og = og_pool.tile([128, CT, 512], BF16, tag="og",
                                  name=f"{R}og{m}")
                for ct in range(CT):
                    nc.sync.dma_start(
                        out=og[:, ct, :],
                        in_=ag_out[m, ct * 128:(ct + 1) * 128, :])
                for ot in range(2):
                    po = aux_pool.tile([128, 512], F32, tag="po",
                                       name=f"{R}po{m}_{ot}", bufs=1)
                    for ct in range(CT):
                        nc.tensor.matmul(
                            po,
                            wo_sb[:, ct, ot * 128:(ot + 1) * 128],
                            og[:, ct, :],
                            start=(ct == 0), stop=(ct == CT - 1))
                    osb = osb_pool.tile([128, 512], F32, tag="osb",
                                        name=f"{R}osb{m}_{ot}")
                    nc.scalar.activation(
                        osb, po, AF.Identity, bias=bo_sb[:, ot:ot + 1])
                    nc.sync.dma_start(
                        out=env["outT_d"][
                            ot * 128:(ot + 1) * 128, m * 512:(m + 1) * 512],
                        in_=osb)

        if dbg:
            nc.sync.dma_start(out=dbg["qT_dbg"][:, :, :], in_=qT.bitcast(F32))
            nc.sync.dma_start(out=dbg["kT_dbg"][:, :, :], in_=kT.bitcast(F32))
            if stat_at:
                nc.sync.dma_start(out=dbg["v_dbg"][:, :, :, :], in_=v_sb)
            nc.gpsimd.dma_start(out=dbg["agin_dbg"][:, :, :], in_=ag_in)
            nc.gpsimd.dma_start(out=dbg["agout_dbg"][:, :, :], in_=ag_out)


def _make_in_maps(queries, keys, values, Wq, bq, Wk, bk, Wv, bv, Wo, bo):
    import ml_dtypes

    bf16 = ml_dtypes.bfloat16
    # bv folds through attention (softmax weights sum to 1) and the output
    # projection into an effective output bias; bk shifts every logit in a
    # row equally so softmax cancels it.
    bo_eff = bo + Wo @ bv
    c = np.ascontiguousarray
    in_maps = []
    for core in range(NCORES):
        b, r = core // 4, core % 4
        sl = slice(r * PC, (r + 1) * PC)
        in_maps.append(
            {
                "xqT": c(queries[b].T),
                "xkT": c(keys[b].T),
                "xvT": c(values[b].T).astype(bf16),
                "wq": c(Wq[sl, :].T),
                "wk": c(Wk[sl, :].T),
                "wv": c(Wv[sl, :].T).astype(bf16),
                "wo": c(Wo.T[:, sl]).astype(bf16),
                "bq": c(bq[sl].reshape(PC, 1)),
                "bo": c(bo_eff[sl].reshape(PC, 1)),
            }
        )
    return in_maps


def kernel(queries, keys, values, Wq, bq, Wk, bk, Wv, bv, Wo, bo, _trace=False):
    import concourse.bass_utils as bass_utils

    args = [queries, keys, values, Wq, bq, Wk, bk, Wv, bv, Wo, bo]
    args = [np.asarray(a, dtype=np.float32) for a in args]

    if "nc" not in _CACHE:
        _CACHE["nc"] = _build()
    nc = _CACHE["nc"]

    in_maps = _make_in_maps(*args)
    res = bass_utils.run_bass_kernel_spmd(
        nc, in_maps, core_ids=list(range(NCORES)), trace=_trace
    )
    _CACHE["last_result"] = res

    out = np.empty((B, M, D), dtype=np.float32)
    for core in range(NCORES):
        b, r = core // 4, core % 4
        out[b, :, r * PC:(r + 1) * PC] = res.results[core]["outT"].T
    return out


# revision 8
# speedup vs baseline: 1.3703x; 1.3703x over previous
"""Multi-head attention (B=2, M=N=2048, D=1024, H=16, DH=64) on 8 TRN2 cores.

Sharding: data-parallel over batch (cores 0-3 = batch 0, 4-7 = batch 1),
tensor-parallel over heads within each batch group (4 heads/core).

v2 design (vs v1): x is transposed on the HOST (kills all PE transposes
and their PSUM->SBUF copies), stage 1 streams x in 512-column blocks on
three DMA queues (sync=xk, vector=xq, gpsimd=weights+xv) so the first
S^T lands ~13us in; stage-1 projection work (kT blocks 1-3, qT m1-3, all
V projections) is interleaved into the (m0,p0) attention loop to fill PE
slack under the ScalarE exp stream.

Attention per (m-chunk 512, head-pair p, n-tile): S^T = K_h Q_h^T row-
tiled pair into one 2-bank PSUM tile; one exp -> bf16 `at` (A^T layout);
AV in one of two dataflows:
  - stat_at: at chunks [n128, m128] are the matmul stationary, moving =
    v_sb[n128, 65] = [V_h | ones] bf16 -> O[m-part, dh] accumulates in
    PSUM with the softmax denominator in col 64. 65-row moving streams
    (bf16 is 1 cycle/row at any free size) halve AV PE rows vs v1;
    normalize is a per-partition DVE reciprocal + tensor_scalar multiply
    (no gpsimd broadcast); O is PE-transposed (bf16 identity) to O^T
    before the AllGather.
  - stat_v: v1 dataflow ([ones|0|V] stationary, at moving, O^T direct).
exp can optionally be split ScalarE/DVE: DVE computes a Schraudolph-
style exp (i16 = round(x*184.665 + B); bitcast bf16) in one
tensor_scalar pass - relative sawtooth error ~3% on attention weights,
which cancels to <1% after softmax normalization + output projection.

Per-m-chunk (256,512) bf16 O^T shards AllGather across the 4-core batch
group; each core computes a 256-wide output-channel slice of out^T =
Wo_slice^T.T @ O^T_full per m-chunk (emitted last => lowest PE priority,
fills PE gaps). Host-side prep: weights pre-transposed/sliced per core,
bv folded into bo_eff = bo + Wo @ bv, bk dropped (cancels in softmax).
"""

import os

import numpy as np

B, M, NSEQ, D = 2, 2048, 2048, 1024
H, DH = 16, 64
HC = 4                # heads per core
PC = HC * DH          # 256 projected channels per core
CT = D // 128         # 8 contraction tiles
NT = NSEQ // 128      # 16 n-tiles
MT = M // 512         # 4 m-chunks
NB = 4                # 512-col x blocks
NCORES = 8

AV_MODE = os.environ.get("AV_MODE", "stat_at")       # stat_at | stat_v
EXP_SPLIT = float(os.environ.get("EXP_SPLIT", "0"))  # fraction of exp tiles on DVE
SCHRAUDOLPH_A = 184.6650
SCHRAUDOLPH_B = float(os.environ.get("SCHRAUDOLPH_B", "16249.0"))

_CACHE = {}


def _build(single_core=False, reps=1, av_mode=None, exp_split=None):
    import concourse.bass as bass
    import concourse.tile as tile
    from concourse import bacc, mybir
    from concourse.masks import make_identity

    av_mode = AV_MODE if av_mode is None else av_mode
    exp_split = EXP_SPLIT if exp_split is None else exp_split

    F32 = mybir.dt.float32
    F32R = mybir.dt.float32r
    BF16 = mybir.dt.bfloat16
    I16 = mybir.dt.int16
    AF = mybir.ActivationFunctionType

    nc = bacc.Bacc(
        "TRN2",
        target_bir_lowering=False,
        debug=False,
        num_devices=1 if single_core else 8,
    )

    xqT_d = nc.dram_tensor("xqT", [D, M], F32R, kind="ExternalInput")
    xkT_d = nc.dram_tensor("xkT", [D, NSEQ], F32R, kind="ExternalInput")
    xvT_d = nc.dram_tensor("xvT", [D, NSEQ], BF16, kind="ExternalInput")
    wq_d = nc.dram_tensor("wq", [D, PC], F32R, kind="ExternalInput")
    wk_d = nc.dram_tensor("wk", [D, PC], F32R, kind="ExternalInput")
    wv_d = nc.dram_tensor("wv", [D, PC], BF16, kind="ExternalInput")
    wo_d = nc.dram_tensor("wo", [D, PC], BF16, kind="ExternalInput")
    bq_d = nc.dram_tensor("bq", [PC, 1], F32, kind="ExternalInput")
    bo_d = nc.dram_tensor("bo", [PC, 1], F32, kind="ExternalInput")
    outT_d = nc.dram_tensor("outT", [PC, M], F32, kind="ExternalOutput")
    debug = bool(int(os.environ.get("KERNEL_DEBUG", "0")))
    dbg = None
    if debug:
        dbg = dict(
            qT_dbg=nc.dram_tensor("qT_dbg", [128, 2, M], F32, kind="ExternalOutput"),
            kT_dbg=nc.dram_tensor("kT_dbg", [128, 2, NSEQ], F32, kind="ExternalOutput"),
            v_dbg=nc.dram_tensor("v_dbg", [128, NT, HC, 65], BF16, kind="ExternalOutput"),
            agin_dbg=nc.dram_tensor("agin_dbg", [MT, PC, 512], BF16, kind="ExternalOutput"),
            agout_dbg=nc.dram_tensor("agout_dbg", [MT, 4 * PC, 512], BF16, kind="ExternalOutput"),
        )

    with tile.TileContext(nc) as tc:
        with (
            tc.tile_pool(name="singles", bufs=1) as singles,
            tc.tile_pool(name="dram", bufs=1, space="DRAM") as dram,
        ):
            ident = singles.tile([128, 128], BF16)
            make_identity(nc, ident)
            zeros_sb = singles.tile([128, 512], BF16)
            nc.vector.memset(zeros_sb, 0.0)
            bq_sb = singles.tile([128, 2], F32)
            nc.gpsimd.dma_start(
                out=bq_sb, in_=bq_d[:, :].rearrange("(o p) w -> p (o w)", p=128)
            )
            bo_sb = singles.tile([128, 2], F32)
            nc.gpsimd.dma_start(
                out=bo_sb, in_=bo_d[:, :].rearrange("(o p) w -> p (o w)", p=128)
            )

            ag_in = dram.tile([MT, PC, 512], BF16)
            ag_out = dram.tile([MT, 4 * PC, 512], BF16)

            env = dict(
                xqT_d=xqT_d, xkT_d=xkT_d, xvT_d=xvT_d,
                wq_d=wq_d, wk_d=wk_d, wv_d=wv_d, wo_d=wo_d,
                outT_d=outT_d, ident=ident, bq_sb=bq_sb, bo_sb=bo_sb,
                zeros_sb=zeros_sb, ag_in=ag_in, ag_out=ag_out,
            )
            for rep in range(reps):
                _emit_rep(
                    nc, tc, bass, mybir, F32, F32R, BF16, I16, AF,
                    rep, single_core, av_mode, exp_split,
                    dbg if (debug and rep == reps - 1) else None, env,
                )
    nc.compile()
    return nc


def _emit_rep(nc, tc, bass, mybir, F32, F32R, BF16, I16, AF, rep,
              single_core, av_mode, exp_split, dbg, env):
    ident = env["ident"]
    zeros_sb = env["zeros_sb"]
    bq_sb, bo_sb = env["bq_sb"], env["bo_sb"]
    ag_in, ag_out = env["ag_in"], env["ag_out"]
    R = f"r{rep}_"
    stat_at = av_mode == "stat_at"

    with (
        tc.tile_pool(name=f"{R}persist", bufs=1) as persist,
        tc.tile_pool(name=f"{R}ps_s", bufs=2, space="PSUM") as ps_s_pool,
        tc.tile_pool(name=f"{R}ps_av", bufs=1 if stat_at else 2,
                     space="PSUM") as ps_av_pool,
        tc.tile_pool(name=f"{R}at", bufs=6) as at_pool,
        tc.tile_pool(name=f"{R}onat", bufs=8) as onat_pool,
        tc.tile_pool(name=f"{R}otb", bufs=2) as otb_pool,
        tc.tile_pool(name=f"{R}rec", bufs=4) as rec_pool,
        tc.tile_pool(name=f"{R}og", bufs=2) as og_pool,
        tc.tile_pool(name=f"{R}osb", bufs=2) as osb_pool,
    ):
        kT = persist.tile([128, 2, M], F32R, name=f"{R}kT")
        qT = persist.tile([128, 2, M], F32R, name=f"{R}qT")
        wk_sb = persist.tile([128, CT, PC], F32R, name=f"{R}wk_sb")
        wq_sb = persist.tile([128, CT, PC], F32R, name=f"{R}wq_sb")
        wv_sb = persist.tile([128, CT, PC], BF16, name=f"{R}wv_sb")
        wo_sb = persist.tile([128, CT, PC], BF16, name=f"{R}wo_sb")
        if stat_at:
            # moving operand of AV: [V_h | ones] per (nt, head)
            v_sb = persist.tile([128, NT, HC, 65], BF16, name=f"{R}v_sb")
        else:
            # v1 layout: [ones | zeros*63 | V] stationary
            v_sb = persist.tile([128, HC, NT, 128], BF16, name=f"{R}v_sb")

        # ---- input DMAs -------------------------------------------------
        # gpsimd queue: weights (k first), then xv blocks, then wo
        nc.gpsimd.dma_start(
            out=wk_sb, in_=env["wk_d"][:, :].rearrange("(ct p) c -> p ct c", p=128))
        nc.gpsimd.dma_start(
            out=wq_sb, in_=env["wq_d"][:, :].rearrange("(ct p) c -> p ct c", p=128))
        nc.gpsimd.dma_start(
            out=wv_sb, in_=env["wv_d"][:, :].rearrange("(ct p) c -> p ct c", p=128))

        with (
            tc.tile_pool(name=f"{R}xst", bufs=3) as xst_pool,
            tc.tile_pool(name=f"{R}xvst", bufs=2) as xvst_pool,
        ):
            xv_st = []
            for b in range(NB):
                t = xvst_pool.tile([128, CT, 512], BF16, tag="xv",
                                   name=f"{R}xv{b}")
                nc.gpsimd.dma_start(
                    out=t,
                    in_=env["xvT_d"][:, :].rearrange(
                        "(ct p) n -> p ct n", p=128)[:, :, b * 512:(b + 1) * 512])
                xv_st.append(t)
            nc.gpsimd.dma_start(
                out=wo_sb,
                in_=env["wo_d"][:, :].rearrange("(ct p) c -> p ct c", p=128))

            # sync queue: xk blocks; vector queue: xq blocks
            xk_st, xq_st = [], []
            for b in range(NB):
                t = xst_pool.tile([128, CT, 512], F32R, tag="xk",
                                  name=f"{R}xk{b}", bufs=2)
                nc.sync.dma_start(
                    out=t,
                    in_=env["xkT_d"][:, :].rearrange(
                        "(ct p) n -> p ct n", p=128)[:, :, b * 512:(b + 1) * 512])
                xk_st.append(t)
            for b in range(NB):
                t = xst_pool.tile([128, CT, 512], F32R, tag="xq",
                                  name=f"{R}xq{b}", bufs=2)
                nc.scalar.dma_start(
                    out=t,
                    in_=env["xqT_d"][:, :].rearrange(
                        "(ct p) m -> p ct m", p=128)[:, :, b * 512:(b + 1) * 512])
                xq_st.append(t)

            if stat_at:
                nc.gpsimd.memset(v_sb[:, :, :, 64:65], 1.0)
            else:
                nc.gpsimd.memset(v_sb[:, :, :, 0:1], 1.0)
                nc.gpsimd.memset(v_sb[:, :, :, 1:64], 0.0)

            # ---- stage-1 emission helpers ------------------------------
            with tc.tile_pool(name=f"{R}pj", bufs=2, space="PSUM") as pj_pool:

                def proj_qk(dst, w_sb, x_st, b, is_q):
                    for ot in range(2):
                        pj = pj_pool.tile([128, 512], F32, tag="pj",
                                          name=f"{R}pj{b}_{ot}_{int(is_q)}")
                        for ct in range(CT):
                            nc.tensor.matmul(
                                pj,
                                w_sb[:, ct, ot * 128:(ot + 1) * 128],
                                x_st[:, ct, :],
                                start=(ct == 0), stop=(ct == CT - 1))
                        d = dst[:, ot, b * 512:(b + 1) * 512]
                        if is_q:
                            nc.vector.tensor_scalar_add(
                                d, pj, bq_sb[:, ot:ot + 1])
                        else:
                            nc.vector.tensor_copy(d, pj)

                def proj_v(nt):
                    vb, s = nt // 4, nt % 4
                    psv = pj_pool.tile([128, 512], F32, tag="pj",
                                       name=f"{R}psv{nt}")
                    for ct in range(CT):
                        nc.tensor.matmul(
                            psv[:, 0:PC],
                            xv_st[vb][:, ct, s * 128:(s + 1) * 128],
                            wv_sb[:, ct, :],
                            start=(ct == 0), stop=(ct == CT - 1))
                    if stat_at:
                        nc.vector.tensor_copy(
                            v_sb[:, nt, :, 0:64],
                            psv[:, 0:PC].rearrange("p (h d) -> p h d", h=HC))
                    else:
                        nc.vector.tensor_copy(
                            v_sb[:, :, nt, 64:128],
                            psv[:, 0:PC].rearrange("p (h d) -> p h d", h=HC))

                # ---- stage-2 emission helpers --------------------------
                def st_exp(m, p, nt, on_dve):
                    ps_s = ps_s_pool.tile([128, 1024], F32, tag="pss",
                                          name=f"{R}pss{m}_{p}_{nt}")
                    for j in range(2):
                        base = j * 64
                        nc.tensor.matmul(
                            ps_s[:, j * 512:(j + 1) * 512],
                            kT[base:base + 64, p, nt * 128:(nt + 1) * 128],
                            qT[base:base + 64, p, m * 512:(m + 1) * 512],
                            start=True, stop=True)
                    at = at_pool.tile([128, 1024], BF16, tag="at",
                                      name=f"{R}at{m}_{p}_{nt}")
                    if on_dve:
                        nc.vector.tensor_scalar(
                            at.bitcast(I16), ps_s,
                            SCHRAUDOLPH_A, SCHRAUDOLPH_B,
                            mybir.AluOpType.mult, mybir.AluOpType.add)
                    else:
                        nc.scalar.activation(at, ps_s, AF.Exp)
                    return at

                def av_zero(av):
                    # a start=True matmul resets its whole PSUM bank, so the
                    # 8 interleaved accumulation regions are instead opened by
                    # one explicit zeroing matmul per bank, and every AV
                    # matmul accumulates (start=False).
                    for j in range(2):
                        nc.tensor.matmul(
                            av[:, j * 512:(j + 1) * 512],
                            zeros_sb[:, 0:128], zeros_sb,
                            start=True, stop=False, skip_group_check=True)

                def av_stat_at(m, p, nt, at, av):
                    for j in range(2):
                        for mi in range(4):
                            nc.tensor.matmul(
                                av[:, j * 512 + mi * 65:j * 512 + mi * 65 + 65],
                                at[:, j * 512 + mi * 128:j * 512 + (mi + 1) * 128],
                                v_sb[:, nt, 2 * p + j, :],
                                start=False, stop=(nt == NT - 1),
                                skip_group_check=True)

                def av_stat_v(m, p, nt, at, ps_o):
                    for j in range(2):
                        nc.tensor.matmul(
                            ps_o[j],
                            v_sb[:, 2 * p + j, nt, :],
                            at[:, j * 512:(j + 1) * 512],
                            start=(nt == 0), stop=(nt == NT - 1))

                def normalize_stat_at(m, p, av, onat_m):
                    rec = rec_pool.tile([128, 8], F32, tag="rec",
                                        name=f"{R}rec{m}_{p}")
                    nc.vector.reciprocal(
                        rec[:, :].rearrange("p (j i) -> p j i", j=2),
                        av[:, :].rearrange("p (j w) -> p j w", j=2)[:, :, 64:260:65])
                    for j in range(2):
                        for mi in range(4):
                            nc.vector.tensor_scalar_mul(
                                onat_m[mi][:, (2 * p + j) * 64:(2 * p + j + 1) * 64],
                                av[:, j * 512 + mi * 65:j * 512 + mi * 65 + 64],
                                rec[:, j * 4 + mi:j * 4 + mi + 1])

                def normalize_stat_v(m, p, ps_o):
                    for j in range(2):
                        h = 2 * p + j
                        rec = rec_pool.tile([1, 512], F32, tag="recv",
                                            name=f"{R}rec{m}_{p}_{j}")
                        nc.vector.reciprocal(rec, ps_o[j][0:1, :])
                        rbc = rec_pool.tile([128, 512], F32, tag="rbc",
                                            name=f"{R}rbc{m}_{p}_{j}")
                        nc.gpsimd.partition_broadcast(rbc, rec[0:1, :])
                        osc = rec_pool.tile([64, 512], BF16, tag="osc",
                                            name=f"{R}osc{m}_{p}_{j}")
                        nc.gpsimd.scalar_tensor_tensor(
                            out=osc, in0=ps_o[j][64:128, :], scalar=1.0,
                            in1=rbc[64:128, :],
                            op0=mybir.AluOpType.mult,
                            op1=mybir.AluOpType.mult)
                        nc.sync.dma_start(
                            out=ag_in[m, h * DH:(h + 1) * DH, :], in_=osc)

                def dve_set(m, p):
                    # which nt tiles exp on DVE (evenly spread);
                    # none during (m0,p0) - DVE is busy with stage-1 copies
                    if exp_split <= 0 or (m == 0 and p == 0):
                        return set()
                    d = int(round(NT * exp_split))
                    return {nt for nt in range(NT) if (nt * d) % NT < d}

                # ============ emission ======================================
                proj_qk(kT, wk_sb, xk_st[0], 0, False)
                proj_qk(qT, wq_sb, xq_st[0], 0, True)

                # (m0, p0) with stage-1 work interleaved
                m, p = 0, 0
                if stat_at:
                    av0 = ps_av_pool.tile([128, 1024], F32, tag="av",
                                          name=f"{R}av{m}_{p}")
                    av_zero(av0)
                else:
                    av0 = [ps_av_pool.tile([128, 512], F32, tag="av",
                                           name=f"{R}av{m}_{p}_{j}")
                           for j in range(2)]
                for nt in range(NT):
                    at = st_exp(m, p, nt, False)
                    proj_v(nt)
                    if stat_at:
                        av_stat_at(m, p, nt, at, av0)
                    else:
                        av_stat_v(m, p, nt, at, av0)
                    if nt in (3, 7, 11):
                        b = nt // 4 + 1
                        proj_qk(kT, wk_sb, xk_st[b], b, False)
                for b in range(1, NB):
                    proj_qk(qT, wq_sb, xq_st[b], b, True)

        # pj pool closed; open aux PSUM pool for transposes + out-proj
        with tc.tile_pool(name=f"{R}aux", bufs=1, space="PSUM") as aux_pool:

            def transpose_out(m, onat_m):
                otb = otb_pool.tile([128, 2, 512], BF16, tag="otb",
                                    name=f"{R}otb{m}")
                for mi in range(4):
                    h2 = mi % 2
                    tr = aux_pool.tile([128, 512], BF16, tag="tr",
                                       name=f"{R}tr{m}_{mi // 2}",
                                       bufs=1) if mi % 2 == 0 else tr
                    for g in range(2):
                        nc.tensor.transpose(
                            tr[:, h2 * 256 + g * 128:h2 * 256 + (g + 1) * 128],
                            onat_m[mi][:, g * 128:(g + 1) * 128],
                            ident)
                    nc.vector.tensor_copy(
                        otb[:, :, mi * 128:(mi + 1) * 128],
                        tr[:, h2 * 256:h2 * 256 + 256].rearrange(
                            "p (g w) -> p g w", g=2))
                nc.sync.dma_start(
                    out=ag_in[m].rearrange("(g p) w -> p g w", p=128),
                    in_=otb)

            def gather(m):
                if single_core:
                    for rr in range(4):
                        nc.sync.dma_start(
                            out=ag_out[m, rr * PC:(rr + 1) * PC, :],
                            in_=ag_in[m, :, :])
                else:
                    nc.gpsimd.collective_compute(
                        "AllGather",
                        bass.mybir.AluOpType.bypass,
                        replica_groups=[[0, 1, 2, 3], [4, 5, 6, 7]],
                        ins=[ag_in[m, :, :].opt()],
                        outs=[ag_out[m, :, :].opt()],
                    )

            # finish stage 2
            for m in range(MT):
                onat_m = None
                if stat_at:
                    onat_m = [
                        onat_pool.tile([128, PC], BF16, tag="onat",
                                       name=f"{R}onat{m}_{mi}")
                        for mi in range(4)
                    ]
                for p in range(2):
                    if m == 0 and p == 0:
                        # emitted above inside the pj scope; just normalize
                        if stat_at:
                            normalize_stat_at(m, p, av0, onat_m)
                        else:
                            normalize_stat_v(m, p, av0)
                        continue
                    if stat_at:
                        av = ps_av_pool.tile([128, 1024], F32, tag="av",
                                             name=f"{R}av{m}_{p}")
                        av_zero(av)
                    else:
                        av = [ps_av_pool.tile([128, 512], F32, tag="av",
                                              name=f"{R}av{m}_{p}_{j}")
                              for j in range(2)]
                    dset = dve_set(m, p)
                    for nt in range(NT):
                        at = st_exp(m, p, nt, nt in dset)
                        if stat_at:
                            av_stat_at(m, p, nt, at, av)
                        else:
                            av_stat_v(m, p, nt, at, av)
                    if stat_at:
                        normalize_stat_at(m, p, av, onat_m)
                    else:
                        normalize_stat_v(m, p, av)
                if stat_at:
                    transpose_out(m, onat_m)
                gather(m)

            # stage 3: output projection (emitted last => lowest PE priority)
            for m in range(MT):
                og = og_pool.tile([128, CT, 512], BF16, tag="og",
                                  name=f"{R}og{m}")
                for ct in range(CT):
                    nc.sync.dma_start(
                        out=og[:, ct, :],
                        in_=ag_out[m, ct * 128:(ct + 1) * 128, :])
                for ot in range(2):
                    po = aux_pool.tile([128, 512], F32, tag="po",
                                       name=f"{R}po{m}_{ot}", bufs=1)
                    for ct in range(CT):
                        nc.tensor.matmul(
                            po,
                            wo_sb[:, ct, ot * 128:(ot + 1) * 128],
                            og[:, ct, :],
                            start=(ct == 0), stop=(ct == CT - 1))
                    osb = osb_pool.tile([128, 512], F32, tag="osb",
                                        name=f"{R}osb{m}_{ot}")
                    nc.scalar.activation(
                        osb, po, AF.Identity, bias=bo_sb[:, ot:ot + 1])
                    nc.sync.dma_start(
                        out=env["outT_d"][
                            ot * 128:(ot + 1) * 128, m * 512:(m + 1) * 512],
                        in_=osb)

        if dbg:
            nc.sync.dma_start(out=dbg["qT_dbg"][:, :, :], in_=qT.bitcast(F32))
            nc.sync.dma_start(out=dbg["kT_dbg"][:, :, :], in_=kT.bitcast(F32))
            if stat_at:
                nc.sync.dma_start(out=dbg["v_dbg"][:, :, :, :], in_=v_sb)
            nc.gpsimd.dma_start(out=dbg["agin_dbg"][:, :, :], in_=ag_in)
            nc.gpsimd.dma_start(out=dbg["agout_dbg"][:, :, :], in_=ag_out)


def _make_in_maps(queries, keys, values, Wq, bq, Wk, bk, Wv, bv, Wo, bo):
    import ml_dtypes

    bf16 = ml_dtypes.bfloat16
    # bv folds through attention (softmax weights sum to 1) and the output
    # projection into an effective output bias; bk shifts every logit in a
    # row equally so softmax cancels it.
    bo_eff = bo + Wo @ bv
    c = np.ascontiguousarray
    in_maps = []
    for core in range(NCORES):
        b, r = core // 4, core % 4
        sl = slice(r * PC, (r + 1) * PC)
        in_maps.append(
            {
                "xqT": c(queries[b].T),
                "xkT": c(keys[b].T),
                "xvT": c(values[b].T).astype(bf16),
                "wq": c(Wq[sl, :].T),
                "wk": c(Wk[sl, :].T),
                "wv": c(Wv[sl, :].T).astype(bf16),
                "wo": c(Wo.T[:, sl]).astype(bf16),
                "bq": c(bq[sl].reshape(PC, 1)),
                "bo": c(bo_eff[sl].reshape(PC, 1)),
            }
        )
    return in_maps


def kernel(queries, keys, values, Wq, bq, Wk, bk, Wv, bv, Wo, bo, _trace=False):
    import concourse.bass_utils as bass_utils

    args = [queries, keys, values, Wq, bq, Wk, bk, Wv, bv, Wo, bo]
    args = [np.asarray(a, dtype=np.float32) for a in args]

    if "nc" not in _CACHE:
        _CACHE["nc"] = _build()
    nc = _CACHE["nc"]

    in_maps = _make_in_maps(*args)
    res = bass_utils.run_bass_kernel_spmd(
        nc, in_maps, core_ids=list(range(NCORES)), trace=_trace
    )
    _CACHE["last_result"] = res

    out = np.empty((B, M, D), dtype=np.float32)
    for core in range(NCORES):
        b, r = core // 4, core % 4
        out[b, :, r * PC:(r + 1) * PC] = res.results[core]["outT"].T
    return out


# revision 9
# speedup vs baseline: 1.5982x; 1.1664x over previous
"""Multi-head attention (B=2, M=N=2048, D=1024, H=16, DH=64) on 8 TRN2 cores.

Sharding: data-parallel over batch (cores 0-3 = batch 0, 4-7 = batch 1),
tensor-parallel over heads within each batch group (4 heads/core).

v2 design (vs v1): x is transposed on the HOST (kills all PE transposes
and their PSUM->SBUF copies), stage 1 streams x in 512-column blocks on
three DMA queues (sync=xk, vector=xq, gpsimd=weights+xv) so the first
S^T lands ~13us in; stage-1 projection work (kT blocks 1-3, qT m1-3, all
V projections) is interleaved into the (m0,p0) attention loop to fill PE
slack under the ScalarE exp stream.

Attention per (m-chunk 512, head-pair p, n-tile): S^T = K_h Q_h^T row-
tiled pair into one 2-bank PSUM tile; one exp -> bf16 `at` (A^T layout);
AV in one of two dataflows:
  - stat_at: at chunks [n128, m128] are the matmul stationary, moving =
    v_sb[n128, 65] = [V_h | ones] bf16 -> O[m-part, dh] accumulates in
    PSUM with the softmax denominator in col 64. 65-row moving streams
    (bf16 is 1 cycle/row at any free size) halve AV PE rows vs v1;
    normalize is a per-partition DVE reciprocal + tensor_scalar multiply
    (no gpsimd broadcast); O is PE-transposed (bf16 identity) to O^T
    before the AllGather.
  - stat_v: v1 dataflow ([ones|0|V] stationary, at moving, O^T direct).
exp can optionally be split ScalarE/DVE: DVE computes a Schraudolph-
style exp (i16 = round(x*184.665 + B); bitcast bf16) in one
tensor_scalar pass - relative sawtooth error ~3% on attention weights,
which cancels to <1% after softmax normalization + output projection.

Per-m-chunk (256,512) bf16 O^T shards AllGather across the 4-core batch
group; each core computes a 256-wide output-channel slice of out^T =
Wo_slice^T.T @ O^T_full per m-chunk (emitted last => lowest PE priority,
fills PE gaps). Host-side prep: weights pre-transposed/sliced per core,
bv folded into bo_eff = bo + Wo @ bv, bk dropped (cancels in softmax).
"""

import os

import numpy as np

B, M, NSEQ, D = 2, 2048, 2048, 1024
H, DH = 16, 64
HC = 4                # heads per core
PC = HC * DH          # 256 projected channels per core
CT = D // 128         # 8 contraction tiles
NT = NSEQ // 128      # 16 n-tiles
MT = M // 512         # 4 m-chunks
NB = 4                # 512-col x blocks
NCORES = 8

AV_MODE = os.environ.get("AV_MODE", "stat_at")          # stat_at | stat_v
EXP_SPLIT = float(os.environ.get("EXP_SPLIT", "0.45"))  # fraction of exp tiles on DVE
SCHRAUDOLPH_A = 184.6650
SCHRAUDOLPH_B = float(os.environ.get("SCHRAUDOLPH_B", "16249.0"))

_CACHE = {}


def _build(single_core=False, reps=1, av_mode=None, exp_split=None):
    import concourse.bass as bass
    import concourse.tile as tile
    from concourse import bacc, mybir
    from concourse.masks import make_identity

    av_mode = AV_MODE if av_mode is None else av_mode
    exp_split = EXP_SPLIT if exp_split is None else exp_split

    F32 = mybir.dt.float32
    F32R = mybir.dt.float32r
    BF16 = mybir.dt.bfloat16
    I16 = mybir.dt.int16
    AF = mybir.ActivationFunctionType

    nc = bacc.Bacc(
        "TRN2",
        target_bir_lowering=False,
        debug=False,
        num_devices=1 if single_core else 8,
    )

    xqT_d = nc.dram_tensor("xqT", [D, M], F32R, kind="ExternalInput")
    xkT_d = nc.dram_tensor("xkT", [D, NSEQ], F32R, kind="ExternalInput")
    xvT_d = nc.dram_tensor("xvT", [D, NSEQ], BF16, kind="ExternalInput")
    wq_d = nc.dram_tensor("wq", [D, PC], F32R, kind="ExternalInput")
    wk_d = nc.dram_tensor("wk", [D, PC], F32R, kind="ExternalInput")
    wv_d = nc.dram_tensor("wv", [D, PC], BF16, kind="ExternalInput")
    wo_d = nc.dram_tensor("wo", [D, PC], BF16, kind="ExternalInput")
    bq_d = nc.dram_tensor("bq", [PC, 1], F32, kind="ExternalInput")
    bo_d = nc.dram_tensor("bo", [PC, 1], F32, kind="ExternalInput")
    outT_d = nc.dram_tensor("outT", [PC, M], F32, kind="ExternalOutput")
    debug = bool(int(os.environ.get("KERNEL_DEBUG", "0")))
    dbg = None
    if debug:
        dbg = dict(
            qT_dbg=nc.dram_tensor("qT_dbg", [128, 2, M], F32, kind="ExternalOutput"),
            kT_dbg=nc.dram_tensor("kT_dbg", [128, 2, NSEQ], F32, kind="ExternalOutput"),
            v_dbg=nc.dram_tensor("v_dbg", [128, NT, HC, 65], BF16, kind="ExternalOutput"),
            agin_dbg=nc.dram_tensor("agin_dbg", [MT, PC, 512], BF16, kind="ExternalOutput"),
            agout_dbg=nc.dram_tensor("agout_dbg", [MT, 4 * PC, 512], BF16, kind="ExternalOutput"),
        )

    with tile.TileContext(nc) as tc:
        with (
            tc.tile_pool(name="singles", bufs=1) as singles,
            tc.tile_pool(name="dram", bufs=1, space="DRAM") as dram,
        ):
            ident = singles.tile([128, 128], BF16)
            make_identity(nc, ident)
            zeros_sb = singles.tile([128, 512], BF16)
            nc.vector.memset(zeros_sb, 0.0)
            bq_sb = singles.tile([128, 2], F32)
            nc.gpsimd.dma_start(
                out=bq_sb, in_=bq_d[:, :].rearrange("(o p) w -> p (o w)", p=128)
            )
            bo_sb = singles.tile([128, 2], F32)
            nc.gpsimd.dma_start(
                out=bo_sb, in_=bo_d[:, :].rearrange("(o p) w -> p (o w)", p=128)
            )

            ag_in = dram.tile([MT, PC, 512], BF16)
            ag_out = dram.tile([MT, 4 * PC, 512], BF16)

            env = dict(
                xqT_d=xqT_d, xkT_d=xkT_d, xvT_d=xvT_d,
                wq_d=wq_d, wk_d=wk_d, wv_d=wv_d, wo_d=wo_d,
                outT_d=outT_d, ident=ident, bq_sb=bq_sb, bo_sb=bo_sb,
                zeros_sb=zeros_sb, ag_in=ag_in, ag_out=ag_out,
            )
            for rep in range(reps):
                _emit_rep(
                    nc, tc, bass, mybir, F32, F32R, BF16, I16, AF,
                    rep, single_core, av_mode, exp_split,
                    dbg if (debug and rep == reps - 1) else None, env,
                )
    nc.compile()
    return nc


def _emit_rep(nc, tc, bass, mybir, F32, F32R, BF16, I16, AF, rep,
              single_core, av_mode, exp_split, dbg, env):
    ident = env["ident"]
    zeros_sb = env["zeros_sb"]
    bq_sb, bo_sb = env["bq_sb"], env["bo_sb"]
    ag_in, ag_out = env["ag_in"], env["ag_out"]
    R = f"r{rep}_"
    stat_at = av_mode == "stat_at"

    with (
        tc.tile_pool(name=f"{R}persist", bufs=1) as persist,
        tc.tile_pool(name=f"{R}ps_s", bufs=2, space="PSUM") as ps_s_pool,
        tc.tile_pool(name=f"{R}ps_av", bufs=1 if stat_at else 2,
                     space="PSUM") as ps_av_pool,
        tc.tile_pool(name=f"{R}at", bufs=6) as at_pool,
        tc.tile_pool(name=f"{R}onat", bufs=8) as onat_pool,
        tc.tile_pool(name=f"{R}otb", bufs=2) as otb_pool,
        tc.tile_pool(name=f"{R}rec", bufs=4) as rec_pool,
        tc.tile_pool(name=f"{R}og", bufs=2) as og_pool,
        tc.tile_pool(name=f"{R}osb", bufs=2) as osb_pool,
    ):
        kT = persist.tile([128, 2, M], F32R, name=f"{R}kT")
        qT = persist.tile([128, 2, M], F32R, name=f"{R}qT")
        wk_sb = persist.tile([128, CT, PC], F32R, name=f"{R}wk_sb")
        wq_sb = persist.tile([128, CT, PC], F32R, name=f"{R}wq_sb")
        wv_sb = persist.tile([128, CT, PC], BF16, name=f"{R}wv_sb")
        wo_sb = persist.tile([128, CT, PC], BF16, name=f"{R}wo_sb")
        if stat_at:
            # moving operand of AV: [V_h | ones] per (nt, head)
            v_sb = persist.tile([128, NT, HC, 65], BF16, name=f"{R}v_sb")
        else:
            # v1 layout: [ones | zeros*63 | V] stationary
            v_sb = persist.tile([128, HC, NT, 128], BF16, name=f"{R}v_sb")

        # ---- input DMAs -------------------------------------------------
        # gpsimd queue: weights (k first), then xv blocks, then wo
        nc.gpsimd.dma_start(
            out=wk_sb, in_=env["wk_d"][:, :].rearrange("(ct p) c -> p ct c", p=128))
        nc.gpsimd.dma_start(
            out=wq_sb, in_=env["wq_d"][:, :].rearrange("(ct p) c -> p ct c", p=128))
        nc.gpsimd.dma_start(
            out=wv_sb, in_=env["wv_d"][:, :].rearrange("(ct p) c -> p ct c", p=128))

        with (
            tc.tile_pool(name=f"{R}xst", bufs=3) as xst_pool,
            tc.tile_pool(name=f"{R}xvst", bufs=2) as xvst_pool,
        ):
            xv_st = []
            for b in range(NB):
                t = xvst_pool.tile([128, CT, 512], BF16, tag="xv",
                                   name=f"{R}xv{b}")
                nc.gpsimd.dma_start(
                    out=t,
                    in_=env["xvT_d"][:, :].rearrange(
                        "(ct p) n -> p ct n", p=128)[:, :, b * 512:(b + 1) * 512])
                xv_st.append(t)
            nc.gpsimd.dma_start(
                out=wo_sb,
                in_=env["wo_d"][:, :].rearrange("(ct p) c -> p ct c", p=128))

            # sync queue: xk blocks; vector queue: xq blocks
            xk_st, xq_st = [], []
            for b in range(NB):
                t = xst_pool.tile([128, CT, 512], F32R, tag="xk",
                                  name=f"{R}xk{b}", bufs=2)
                nc.sync.dma_start(
                    out=t,
                    in_=env["xkT_d"][:, :].rearrange(
                        "(ct p) n -> p ct n", p=128)[:, :, b * 512:(b + 1) * 512])
                xk_st.append(t)
            for b in range(NB):
                t = xst_pool.tile([128, CT, 512], F32R, tag="xq",
                                  name=f"{R}xq{b}", bufs=2)
                nc.scalar.dma_start(
                    out=t,
                    in_=env["xqT_d"][:, :].rearrange(
                        "(ct p) m -> p ct m", p=128)[:, :, b * 512:(b + 1) * 512])
                xq_st.append(t)

            if stat_at:
                nc.gpsimd.memset(v_sb[:, :, :, 64:65], 1.0)
            else:
                nc.gpsimd.memset(v_sb[:, :, :, 0:1], 1.0)
                nc.gpsimd.memset(v_sb[:, :, :, 1:64], 0.0)

            # ---- stage-1 emission helpers ------------------------------
            with tc.tile_pool(name=f"{R}pj", bufs=2, space="PSUM") as pj_pool:

                def proj_qk(dst, w_sb, x_st, b, is_q):
                    for ot in range(2):
                        pj = pj_pool.tile([128, 512], F32, tag="pj",
                                          name=f"{R}pj{b}_{ot}_{int(is_q)}")
                        for ct in range(CT):
                            nc.tensor.matmul(
                                pj,
                                w_sb[:, ct, ot * 128:(ot + 1) * 128],
                                x_st[:, ct, :],
                                start=(ct == 0), stop=(ct == CT - 1))
                        d = dst[:, ot, b * 512:(b + 1) * 512]
                        if is_q:
                            nc.vector.tensor_scalar_add(
                                d, pj, bq_sb[:, ot:ot + 1])
                        else:
                            nc.vector.tensor_copy(d, pj)

                def proj_v(nt):
                    vb, s = nt // 4, nt % 4
                    psv = pj_pool.tile([128, 512], F32, tag="pj",
                                       name=f"{R}psv{nt}")
                    for ct in range(CT):
                        nc.tensor.matmul(
                            psv[:, 0:PC],
                            xv_st[vb][:, ct, s * 128:(s + 1) * 128],
                            wv_sb[:, ct, :],
                            start=(ct == 0), stop=(ct == CT - 1))
                    if stat_at:
                        nc.vector.tensor_copy(
                            v_sb[:, nt, :, 0:64],
                            psv[:, 0:PC].rearrange("p (h d) -> p h d", h=HC))
                    else:
                        nc.vector.tensor_copy(
                            v_sb[:, :, nt, 64:128],
                            psv[:, 0:PC].rearrange("p (h d) -> p h d", h=HC))

                # ---- stage-2 emission helpers --------------------------
                def st_exp(m, p, nt, on_dve):
                    ps_s = ps_s_pool.tile([128, 1024], F32, tag="pss",
                                          name=f"{R}pss{m}_{p}_{nt}")
                    for j in range(2):
                        base = j * 64
                        nc.tensor.matmul(
                            ps_s[:, j * 512:(j + 1) * 512],
                            kT[base:base + 64, p, nt * 128:(nt + 1) * 128],
                            qT[base:base + 64, p, m * 512:(m + 1) * 512],
                            start=True, stop=True)
                    at = at_pool.tile([128, 1024], BF16, tag="at",
                                      name=f"{R}at{m}_{p}_{nt}")
                    if on_dve:
                        nc.vector.tensor_scalar(
                            at.bitcast(I16), ps_s,
                            SCHRAUDOLPH_A, SCHRAUDOLPH_B,
                            mybir.AluOpType.mult, mybir.AluOpType.add)
                    else:
                        nc.scalar.activation(at, ps_s, AF.Exp)
                    return at

                def av_zero(av):
                    # a start=True matmul resets its whole PSUM bank, so the
                    # 8 interleaved accumulation regions are instead opened by
                    # one explicit zeroing matmul per bank, and every AV
                    # matmul accumulates (start=False).
                    for j in range(2):
                        nc.tensor.matmul(
                            av[:, j * 512:(j + 1) * 512],
                            zeros_sb[:, 0:128], zeros_sb,
                            start=True, stop=False, skip_group_check=True)

                def av_stat_at(m, p, nt, at, av):
                    for j in range(2):
                        for mi in range(4):
                            nc.tensor.matmul(
                                av[:, j * 512 + mi * 65:j * 512 + mi * 65 + 65],
                                at[:, j * 512 + mi * 128:j * 512 + (mi + 1) * 128],
                                v_sb[:, nt, 2 * p + j, :],
                                start=False, stop=(nt == NT - 1),
                                skip_group_check=True)

                def av_stat_v(m, p, nt, at, ps_o):
                    for j in range(2):
                        nc.tensor.matmul(
                            ps_o[j],
                            v_sb[:, 2 * p + j, nt, :],
                            at[:, j * 512:(j + 1) * 512],
                            start=(nt == 0), stop=(nt == NT - 1))

                def normalize_stat_at(m, p, av, onat_m):
                    rec = rec_pool.tile([128, 8], F32, tag="rec",
                                        name=f"{R}rec{m}_{p}")
                    nc.vector.reciprocal(
                        rec[:, :].rearrange("p (j i) -> p j i", j=2),
                        av[:, :].rearrange("p (j w) -> p j w", j=2)[:, :, 64:260:65])
                    for j in range(2):
                        for mi in range(4):
                            nc.vector.tensor_scalar_mul(
                                onat_m[mi][:, (2 * p + j) * 64:(2 * p + j + 1) * 64],
                                av[:, j * 512 + mi * 65:j * 512 + mi * 65 + 64],
                                rec[:, j * 4 + mi:j * 4 + mi + 1])

                def normalize_stat_v(m, p, ps_o):
                    for j in range(2):
                        h = 2 * p + j
                        rec = rec_pool.tile([1, 512], F32, tag="recv",
                                            name=f"{R}rec{m}_{p}_{j}")
                        nc.vector.reciprocal(rec, ps_o[j][0:1, :])
                        rbc = rec_pool.tile([128, 512], F32, tag="rbc",
                                            name=f"{R}rbc{m}_{p}_{j}")
                        nc.gpsimd.partition_broadcast(rbc, rec[0:1, :])
                        osc = rec_pool.tile([64, 512], BF16, tag="osc",
                                            name=f"{R}osc{m}_{p}_{j}")
                        nc.gpsimd.scalar_tensor_tensor(
                            out=osc, in0=ps_o[j][64:128, :], scalar=1.0,
                            in1=rbc[64:128, :],
                            op0=mybir.AluOpType.mult,
                            op1=mybir.AluOpType.mult)
                        nc.sync.dma_start(
                            out=ag_in[m, h * DH:(h + 1) * DH, :], in_=osc)

                def dve_set(m, p):
                    # which nt tiles exp on DVE (evenly spread);
                    # none during (m0,p0) - DVE is busy with stage-1 copies
                    if exp_split <= 0 or (m == 0 and p == 0):
                        return set()
                    d = int(round(NT * exp_split))
                    return {nt for nt in range(NT) if (nt * d) % NT < d}

                # ============ emission ======================================
                proj_qk(kT, wk_sb, xk_st[0], 0, False)
                proj_qk(qT, wq_sb, xq_st[0], 0, True)

                # (m0, p0) with stage-1 work interleaved
                m, p = 0, 0
                if stat_at:
                    av0 = ps_av_pool.tile([128, 1024], F32, tag="av",
                                          name=f"{R}av{m}_{p}")
                    av_zero(av0)
                else:
                    av0 = [ps_av_pool.tile([128, 512], F32, tag="av",
                                           name=f"{R}av{m}_{p}_{j}")
                           for j in range(2)]
                for nt in range(NT):
                    at = st_exp(m, p, nt, False)
                    proj_v(nt)
                    if stat_at:
                        av_stat_at(m, p, nt, at, av0)
                    else:
                        av_stat_v(m, p, nt, at, av0)
                    if nt in (3, 7, 11):
                        b = nt // 4 + 1
                        proj_qk(kT, wk_sb, xk_st[b], b, False)
                for b in range(1, NB):
                    proj_qk(qT, wq_sb, xq_st[b], b, True)

        # pj pool closed; open aux PSUM pool for transposes + out-proj
        with tc.tile_pool(name=f"{R}aux", bufs=1, space="PSUM") as aux_pool:

            def transpose_out(m, onat_m):
                otb = otb_pool.tile([128, 2, 512], BF16, tag="otb",
                                    name=f"{R}otb{m}")
                for mi in range(4):
                    h2 = mi % 2
                    tr = aux_pool.tile([128, 512], BF16, tag="tr",
                                       name=f"{R}tr{m}_{mi // 2}",
                                       bufs=1) if mi % 2 == 0 else tr
                    for g in range(2):
                        nc.tensor.transpose(
                            tr[:, h2 * 256 + g * 128:h2 * 256 + (g + 1) * 128],
                            onat_m[mi][:, g * 128:(g + 1) * 128],
                            ident)
                    nc.vector.tensor_copy(
                        otb[:, :, mi * 128:(mi + 1) * 128],
                        tr[:, h2 * 256:h2 * 256 + 256].rearrange(
                            "p (g w) -> p g w", g=2))
                nc.sync.dma_start(
                    out=ag_in[m].rearrange("(g p) w -> p g w", p=128),
                    in_=otb)

            def gather(m):
                if single_core:
                    for rr in range(4):
                        nc.sync.dma_start(
                            out=ag_out[m, rr * PC:(rr + 1) * PC, :],
                            in_=ag_in[m, :, :])
                else:
                    nc.gpsimd.collective_compute(
                        "AllGather",
                        bass.mybir.AluOpType.bypass,
                        replica_groups=[[0, 1, 2, 3], [4, 5, 6, 7]],
                        ins=[ag_in[m, :, :].opt()],
                        outs=[ag_out[m, :, :].opt()],
                    )

            # finish stage 2
            for m in range(MT):
                onat_m = None
                if stat_at:
                    onat_m = [
                        onat_pool.tile([128, PC], BF16, tag="onat",
                                       name=f"{R}onat{m}_{mi}")
                        for mi in range(4)
                    ]
                for p in range(2):
                    if m == 0 and p == 0:
                        # emitted above inside the pj scope; just normalize
                        if stat_at:
                            normalize_stat_at(m, p, av0, onat_m)
                        else:
                            normalize_stat_v(m, p, av0)
                        continue
                    if stat_at:
                        av = ps_av_pool.tile([128, 1024], F32, tag="av",
                                             name=f"{R}av{m}_{p}")
                        av_zero(av)
                    else:
                        av = [ps_av_pool.tile([128, 512], F32, tag="av",
                                              name=f"{R}av{m}_{p}_{j}")
                              for j in range(2)]
                    dset = dve_set(m, p)
                    for nt in range(NT):
                        at = st_exp(m, p, nt, nt in dset)
                        if stat_at:
                            av_stat_at(m, p, nt, at, av)
                        else:
                            av_stat_v(m, p, nt, at, av)
                    if stat_at:
                        normalize_stat_at(m, p, av, onat_m)
                    else:
                        normalize_stat_v(m, p, av)
                if stat_at:
                    transpose_out(m, onat_m)
                gather(m)

            # stage 3: output projection (emitted last => lowest PE priority)
            for m in range(MT):
                og = og_pool.tile([128, CT, 512], BF16, tag="og",
                                  name=f"{R}og{m}")
                for ct in range(CT):
                    nc.sync.dma_start(
                        out=og[:, ct, :],
                        in_=ag_out[m, ct * 128:(ct + 1) * 128, :])
                for ot in range(2):
                    po = aux_pool.tile([128, 512], F32, tag="po",
                                       name=f"{R}po{m}_{ot}", bufs=1)
                    for ct in range(CT):
                        nc.tensor.matmul(
                            po,
                            wo_sb[:, ct, ot * 128:(ot + 1) * 128],
                            og[:, ct, :],
                            start=(ct == 0), stop=(ct == CT - 1))
                    osb = osb_pool.tile([128, 512], F32, tag="osb",
                                        name=f"{R}osb{m}_{ot}")
                    nc.scalar.activation(
                        osb, po, AF.Identity, bias=bo_sb[:, ot:ot + 1])
                    nc.sync.dma_start(
                        out=env["outT_d"][
                            ot * 128:(ot + 1) * 128, m * 512:(m + 1) * 512],
                        in_=osb)

        if dbg:
            nc.sync.dma_start(out=dbg["qT_dbg"][:, :, :], in_=qT.bitcast(F32))
            nc.sync.dma_start(out=dbg["kT_dbg"][:, :, :], in_=kT.bitcast(F32))
            if stat_at:
                nc.sync.dma_start(out=dbg["v_dbg"][:, :, :, :], in_=v_sb)
            nc.gpsimd.dma_start(out=dbg["agin_dbg"][:, :, :], in_=ag_in)
            nc.gpsimd.dma_start(out=dbg["agout_dbg"][:, :, :], in_=ag_out)


def _make_in_maps(queries, keys, values, Wq, bq, Wk, bk, Wv, bv, Wo, bo):
    import ml_dtypes

    bf16 = ml_dtypes.bfloat16
    # bv folds through attention (softmax weights sum to 1) and the output
    # projection into an effective output bias; bk shifts every logit in a
    # row equally so softmax cancels it.
    bo_eff = bo + Wo @ bv
    c = np.ascontiguousarray
    in_maps = []
    for core in range(NCORES):
        b, r = core // 4, core % 4
        sl = slice(r * PC, (r + 1) * PC)
        in_maps.append(
            {
                "xqT": c(queries[b].T),
                "xkT": c(keys[b].T),
                "xvT": c(values[b].T).astype(bf16),
                "wq": c(Wq[sl, :].T),
                "wk": c(Wk[sl, :].T),
                "wv": c(Wv[sl, :].T).astype(bf16),
                "wo": c(Wo.T[:, sl]).astype(bf16),
                "bq": c(bq[sl].reshape(PC, 1)),
                "bo": c(bo_eff[sl].reshape(PC, 1)),
            }
        )
    return in_maps


def kernel(queries, keys, values, Wq, bq, Wk, bk, Wv, bv, Wo, bo, _trace=False):
    import concourse.bass_utils as bass_utils

    args = [queries, keys, values, Wq, bq, Wk, bk, Wv, bv, Wo, bo]
    args = [np.asarray(a, dtype=np.float32) for a in args]

    if "nc" not in _CACHE:
        _CACHE["nc"] = _build()
    nc = _CACHE["nc"]

    in_maps = _make_in_maps(*args)
    res = bass_utils.run_bass_kernel_spmd(
        nc, in_maps, core_ids=list(range(NCORES)), trace=_trace
    )
    _CACHE["last_result"] = res

    out = np.empty((B, M, D), dtype=np.float32)
    for core in range(NCORES):
        b, r = core // 4, core % 4
        out[b, :, r * PC:(r + 1) * PC] = res.results[core]["outT"].T
    return out
